# revision 1
# baseline (speedup 1.0000x reference)
"""DeepFFM Trainium2 kernel (8 NeuronCores, SPMD via bass/Tile).

Math (reference):
  linear      = X @ w1 + b
  S[i,j]      = <nfk[i, f2f[j], :], nfk[j, f2f[i], :]>   (symmetric, param-only)
  interaction = sum_{i<j} S[i,j] X[:,i] X[:,j] = 0.5 * rowsum((X @ Sz) * X)
                where Sz = S with zeroed diagonal (uses symmetry of S)
  deep        = MLP(X) with relu layers
  out         = linear + interaction + deep

Strategy:
  * Host-side: sort features by field (permutation). All float tensors are
    permuted / transposed / cast to bf16 host-side (layout transforms only;
    all FLOPs on device). In sorted order, S splits into per-(field g, field
    f) rank-40 blocks  S[J_g, J_f] = nfkT[f-rows, J_g]^T @ nfkT[g-rows, J_f]
    which are contiguous slices of nfkT = nfk.reshape(F, FIELDS*K).T.
  * S rows are sharded over the 8 cores (5 field groups per core, group list
    padded to 40 groups). Per-core variation lives entirely in DATA (each
    core's in_map carries its own nfkT column/row slices in a fixed padded
    local layout) so the SPMD instruction stream is identical on all cores.
  * Sharded S blocks -> AllGather (bf16, launched as soon as the S shard is
    written — it overlaps the input DMA stream and the deep MLP) -> each
    core rebuilds the full Sz (compact, diagonal zeroed via affine_select).
  * Batch is sharded 512 rows/core for linear+deep+interaction. Activations
    stay transposed (hT = W^T @ hT_prev) so only XT ([F, batch]) is needed.
  * Everything flows in bf16 (PSUM accumulation is fp32): halves all HBM
    traffic and runs every matmul at the full 1 cycle/row PE rate.
  * DMA queueing: TRN2 has exactly two hardware DGE queues (sync, scalar).
    Bulk loads are split across them by need-time; small / late tensors ride
    gpsimd's software DGE so they never block the critical streams.
"""

import numpy as np

import concourse.bass as bass
import concourse.bacc as bacc
import concourse.mybir as mybir
import concourse.tile as tile
from concourse.tile_rust import add_dep_helper
from concourse.bass_utils import run_bass_kernel_spmd

F32 = mybir.dt.float32
F32R = mybir.dt.float32r
BF16 = mybir.dt.bfloat16
DEEP_DT = BF16    # deep-chain matmul dtype
SDT = BF16        # S phase / AllGather dtype
XDT = BF16        # XT dtype

NCORES = 8
B = 4096
BS = B // NCORES          # batch rows per core
F = 1000                  # feature size
FIELDS = 39
NGT = 40                  # padded group count (group 39 empty)
GPC = NGT // NCORES       # groups per core = 5
K = 40                    # ffm embedding dim
D0, D1, D2 = 1024, 512, 256
KT0 = 8                   # k-chunks (125) over F
P_F = 125                 # partition chunk of F


def _build_program(off, counts, pad, bias_const, sim_single_core=False, repeat=None):
    """Trace the (SPMD-identical) bass program. off/counts describe the
    globally-sorted field layout; pad is the per-group local row pitch."""
    lrows = GPC * pad
    nc = bacc.Bacc(None, num_devices=NCORES)

    # All inputs are host-prearranged to their exact SBUF tile layouts so
    # every load is a single contiguous DMA.
    xt_h = nc.dram_tensor("xt", [P_F, KT0, BS], XDT, kind="ExternalInput")
    w0_h = nc.dram_tensor("w0", [P_F, 2, KT0, D1], DEEP_DT, kind="ExternalInput")
    w1_h = nc.dram_tensor("w1w", [128, 8, D1], DEEP_DT, kind="ExternalInput")
    w2_h = nc.dram_tensor("w2w", [128, 4, D2], DEEP_DT, kind="ExternalInput")
    ow_h = nc.dram_tensor("outw", [128, 2, 1], DEEP_DT, kind="ExternalInput")
    wl_h = nc.dram_tensor("w1lin", [P_F, KT0, 1], XDT, kind="ExternalInput")
    b0_h = nc.dram_tensor("b0", [128, 8], F32, kind="ExternalInput")
    b1_h = nc.dram_tensor("b1", [128, 4], F32, kind="ExternalInput")
    b2_h = nc.dram_tensor("b2", [128, 2], F32, kind="ExternalInput")
    nk_h = nc.dram_tensor("nfkt_cols", [K, FIELDS, lrows], SDT, kind="ExternalInput")
    gt_h = nc.dram_tensor("gtiles", [K, GPC, F], SDT, kind="ExternalInput")
    hv_h = nc.dram_tensor("halves", [P_F, 1], XDT, kind="ExternalInput")
    out_h = nc.dram_tensor("out", [1, BS], F32, kind="ExternalOutput")

    # column segments of each field block, split at PSUM bank (512) boundaries
    def col_segs(f):
        c0, c1 = int(off[f]), int(off[f + 1])
        segs = []
        while c0 < c1:
            nxt = min(c1, (c0 // 512 + 1) * 512)
            segs.append((c0, nxt))
            c0 = nxt
        return segs

    # reload map: AG-out padded rows -> compact global rows, split at 125-tiles
    reload_segs = []
    for g in range(FIELDS):
        c, gl = divmod(g, GPC)
        src = c * lrows + gl * pad
        dst = int(off[g])
        n = int(counts[g])
        while n > 0:
            t = dst // P_F
            po = dst % P_F
            take = min(n, P_F - po)
            reload_segs.append((src, t, po, take))
            src += take
            dst += take
            n -= take

    with tile.TileContext(nc) as tc:
        with (
            tc.tile_pool(name="persist", bufs=1) as persist,
            tc.tile_pool(name="sphase", bufs=1) as sphase,
            tc.tile_pool(name="evac", bufs=2) as evac,
            tc.tile_pool(name="work", bufs=2) as work,
            tc.tile_pool(name="psum", bufs=1, space="PSUM") as psum,
            tc.tile_pool(name="dram", bufs=1, space="DRAM") as dram,
        ):
            import contextlib
            rep_ctx = (tc.For_i(0, repeat, 1) if repeat is not None
                       else contextlib.nullcontext())
            with rep_ctx:
                # ---------------- loads ----------------
                # sync HW queue: S-phase columns first, then the agin stores
                # (issued inside the S loop), then second-layer weights.
                # scalar HW queue: gtiles, then the deep-critical xt/w0 stream.
                # gpsimd SW queue: small and late-needed tensors.
                nfkt_sb = sphase.tile([K, FIELDS, lrows], SDT)
                nc.sync.dma_start(out=nfkt_sb, in_=nk_h[:])
                gt_sb = sphase.tile([K, GPC, F], SDT)
                nc.scalar.dma_start(out=gt_sb, in_=gt_h[:])
                xt_sb = persist.tile([P_F, KT0, BS], XDT)
                nc.scalar.dma_start(out=xt_sb, in_=xt_h[:])
                w0_sb = persist.tile([P_F, 2, KT0, D1], DEEP_DT)
                nc.scalar.dma_start(out=w0_sb[:, 0, :, :], in_=w0_h[:, 0, :, :])
                nc.scalar.dma_start(out=w0_sb[:, 1, :, :], in_=w0_h[:, 1, :, :])

                halves = persist.tile([P_F, 1], XDT)
                nc.gpsimd.dma_start(out=halves, in_=hv_h[:])
                b0_sb = persist.tile([128, 8], F32)
                nc.gpsimd.dma_start(out=b0_sb, in_=b0_h[:])
                b1_sb = persist.tile([128, 4], F32)
                nc.gpsimd.dma_start(out=b1_sb, in_=b1_h[:])
                b2_sb = persist.tile([128, 2], F32)
                nc.gpsimd.dma_start(out=b2_sb, in_=b2_h[:])
                wl_sb = persist.tile([P_F, KT0, 1], XDT)
                nc.gpsimd.dma_start(out=wl_sb, in_=wl_h[:])
                ow_sb = persist.tile([128, 2, 1], DEEP_DT)
                nc.gpsimd.dma_start(out=ow_sb, in_=ow_h[:])
                w2_sb = persist.tile([128, 4, D2], DEEP_DT)
                nc.gpsimd.dma_start(out=w2_sb, in_=w2_h[:])

                # PE warm-up: the HAM clock gate needs sustained PE activity
                # to ramp.  Burn the initial DMA-wait window with dummy
                # matmuls on a memset scratch tile.
                warm_sb = work.tile([128, 128], BF16, tag="warm", bufs=1)
                nc.vector.memset(warm_sb, 1.0)
                ps_w = psum.tile([128, 64], F32, tag="ps_o", bufs=1)
                for _ in range(16):
                    nc.tensor.matmul(
                        ps_w, lhsT=warm_sb[:, 0:128], rhs=warm_sb[:, 0:64],
                        start=True, stop=True,
                    )

                agin = dram.tile([lrows, F], SDT)
                agout = dram.tile(
                    [NCORES * lrows, F], SDT,
                    addr_space="Local" if sim_single_core else "Shared",
                )

                # ---------------- S phase: per-group block matmuls ----------------
                for gl in range(GPC):
                    ps_s = psum.tile([pad, F], F32, tag="ps_s", bufs=2)
                    for f in range(FIELDS):
                        for (c0, c1) in col_segs(f):
                            nc.tensor.matmul(
                                ps_s[:, c0:c1],
                                lhsT=nfkt_sb[:, f, gl * pad : (gl + 1) * pad],
                                rhs=gt_sb[:, gl, c0:c1],
                                start=True,
                                stop=True,
                            )
                    srow = evac.tile([pad, F], SDT, tag="srow")
                    nc.vector.tensor_copy(srow, ps_s)
                    nc.sync.dma_start(out=agin[gl * pad : (gl + 1) * pad, :], in_=srow)

                w1_sb = persist.tile([128, 8, D1], DEEP_DT)
                nc.sync.dma_start(out=w1_sb, in_=w1_h[:])

                if sim_single_core:
                    # Timeline-sim stand-in for the AllGather (single-core cost
                    # model can't simulate collectives): copy the shard into all 8
                    # rank slots — writes every agout byte (correct deps for the
                    # reload DMAs) and costs ~the real AG wire time.
                    for r in range(NCORES):
                        nc.sync.dma_start(
                            out=agout[r * lrows : (r + 1) * lrows, :], in_=agin[:]
                        )
                else:
                    nc.gpsimd.collective_compute(
                        "AllGather",
                        mybir.AluOpType.bypass,
                        replica_groups=[list(range(NCORES))],
                        ins=[agin[:].opt()],
                        outs=[agout[:].opt()],
                    )

                # keep the HAM clock warm across the xt/w0 DMA-wait gap
                for _ in range(10):
                    nc.tensor.matmul(
                        ps_w, lhsT=warm_sb[:, 0:128], rhs=warm_sb[:, 0:64],
                        start=True, stop=True,
                    )

                # ---------------- deep MLP (overlaps the collective) -------------
                h0_sb = persist.tile([128, 8, D1], BF16)
                ps_o = psum.tile([1, BS], F32, tag="ps_o", bufs=1)
                for mj in range(8):
                    ps0 = psum.tile([128, BS], F32, tag="ps_mm", bufs=3)
                    for t in range(KT0):
                        nc.tensor.matmul(
                            ps0,
                            lhsT=w0_sb[:, mj // 4, t, (mj % 4) * 128 : (mj % 4 + 1) * 128],
                            rhs=xt_sb[:, t, :],
                            start=(t == 0),
                            stop=(t == KT0 - 1),
                        )
                    nc.scalar.activation(
                        h0_sb[:, mj, :],
                        ps0,
                        mybir.ActivationFunctionType.Relu,
                        bias=b0_sb[:, mj : mj + 1],
                    )
                h1_sb = persist.tile([128, 4, BS], BF16)
                for mj in range(4):
                    ps1 = psum.tile([128, BS], F32, tag="ps_mm", bufs=3)
                    for t in range(8):
                        nc.tensor.matmul(
                            ps1,
                            lhsT=w1_sb[:, t, mj * 128 : (mj + 1) * 128],
                            rhs=h0_sb[:, t, :],
                            start=(t == 0),
                            stop=(t == 7),
                        )
                    nc.scalar.activation(
                        h1_sb[:, mj, :],
                        ps1,
                        mybir.ActivationFunctionType.Relu,
                        bias=b1_sb[:, mj : mj + 1],
                    )
                h2_sb = persist.tile([128, 2, BS], BF16)
                for mj in range(2):
                    ps2 = psum.tile([128, BS], F32, tag="ps_mm", bufs=3)
                    for t in range(4):
                        nc.tensor.matmul(
                            ps2,
                            lhsT=w2_sb[:, t, mj * 128 : (mj + 1) * 128],
                            rhs=h1_sb[:, t, :],
                            start=(t == 0),
                            stop=(t == 3),
                        )
                    nc.scalar.activation(
                        h2_sb[:, mj, :],
                        ps2,
                        mybir.ActivationFunctionType.Relu,
                        bias=b2_sb[:, mj : mj + 1],
                    )
                # ps_o accumulation group: deep head + linear + interaction
                for t in range(2):
                    nc.tensor.matmul(
                        ps_o,
                        lhsT=ow_sb[:, t, :],
                        rhs=h2_sb[:, t, :],
                        start=(t == 0),
                        stop=False,
                    )
                for t in range(KT0):
                    nc.tensor.matmul(
                        ps_o,
                        lhsT=wl_sb[:, t, :],
                        rhs=xt_sb[:, t, :],
                        start=False,
                        stop=False,
                    )

                # ---------------- rebuild full Sz from the AllGather -------------
                s_sb = persist.tile([P_F, KT0, F], SDT)
                engs = [nc.sync, nc.scalar]
                for i, (src, t, po, n) in enumerate(reload_segs):
                    engs[i % len(engs)].dma_start(
                        out=s_sb[po : po + n, t, :], in_=agout[src : src + n, :]
                    )
                for t in range(KT0):
                    nc.gpsimd.affine_select(
                        out=s_sb[:, t, t * P_F : (t + 1) * P_F],
                        in_=s_sb[:, t, t * P_F : (t + 1) * P_F],
                        compare_op=mybir.AluOpType.not_equal,
                        fill=0.0,
                        base=0,
                        pattern=[[-1, P_F]],
                        channel_multiplier=1,
                    )

                # ---------------- interaction: YT = Sz @ XT, 0.5*colsum(YT*XT) ---
                for mj in range(KT0):
                    ps_y = psum.tile([P_F, BS], F32, tag="ps_mm", bufs=3)
                    for t in range(KT0):
                        nc.tensor.matmul(
                            ps_y,
                            lhsT=s_sb[:, t, mj * P_F : (mj + 1) * P_F],
                            rhs=xt_sb[:, t, :],
                            start=(t == 0),
                            stop=(t == KT0 - 1),
                        )
                    z_sb = work.tile([P_F, BS], XDT, tag="z")
                    nc.vector.tensor_mul(z_sb, ps_y, xt_sb[:, mj, :])
                    nc.tensor.matmul(
                        ps_o,
                        lhsT=halves,
                        rhs=z_sb,
                        start=False,
                        stop=(mj == KT0 - 1),
                    )

                # ---------------- final: add folded scalar bias, store -----------
                out_sb = persist.tile([1, BS], F32)
                nc.vector.tensor_scalar_add(out_sb, ps_o, float(bias_const))
                nc.sync.dma_start(out=out_h[:], in_=out_sb)

    nc.compile()
    return nc


def kernel(X, w1, b, nfk, f2f, deepW0, deepB0, deepW1, deepB1, deepW2, deepB2,
           outW, outB, **_unused):
    import ml_dtypes
    bf16 = ml_dtypes.bfloat16

    X = np.ascontiguousarray(X, dtype=np.float32)
    w1 = np.asarray(w1, dtype=np.float32)
    b = np.asarray(b, dtype=np.float32)
    nfk = np.ascontiguousarray(nfk, dtype=np.float32)
    f2f = np.asarray(f2f)
    deepW0 = np.ascontiguousarray(deepW0, dtype=np.float32)
    deepW1 = np.ascontiguousarray(deepW1, dtype=np.float32)
    deepW2 = np.ascontiguousarray(deepW2, dtype=np.float32)
    outW = np.ascontiguousarray(outW, dtype=np.float32)

    # ---- host-side layout transforms (index/permutation/cast work only) ----
    perm = np.argsort(f2f, kind="stable")
    counts = np.bincount(np.asarray(f2f, dtype=np.int64), minlength=NGT)[:NGT]
    off = np.zeros(NGT + 1, dtype=np.int64)
    off[1:] = np.cumsum(counts)
    pad = int(max(counts.max(), 1))
    lrows = GPC * pad

    XT = np.ascontiguousarray(X[:, perm].T)                     # [F, B]
    w1p = np.ascontiguousarray(w1[perm].reshape(F, 1))
    nfkp = nfk[perm]                                            # [F, FIELDS, K]
    nfkT = np.ascontiguousarray(nfkp.reshape(F, FIELDS * K).T)  # [FIELDS*K, F]
    W0p = np.ascontiguousarray(deepW0[perm])
    bias_const = float(np.float32(b[0]) + np.float32(outB[0]))

    nc = _build_program(off, counts, pad, bias_const)

    def _c(a, dt=bf16):
        return np.ascontiguousarray(a).astype(dt)

    w0_dev = _c(W0p.reshape(KT0, P_F, 2, D1).transpose(1, 2, 0, 3))
    w1_dev = _c(deepW1.reshape(8, 128, D1).transpose(1, 0, 2))
    w2_dev = _c(deepW2.reshape(4, 128, D2).transpose(1, 0, 2))
    ow_dev = _c(outW.reshape(2, 128, 1).transpose(1, 0, 2))
    wl_dev = _c(w1p.reshape(KT0, P_F, 1).transpose(1, 0, 2))
    b0_dev = np.ascontiguousarray(np.asarray(deepB0, np.float32).reshape(8, 128).T)
    b1_dev = np.ascontiguousarray(np.asarray(deepB1, np.float32).reshape(4, 128).T)
    b2_dev = np.ascontiguousarray(np.asarray(deepB2, np.float32).reshape(2, 128).T)
    halves_dev = np.full((P_F, 1), 0.5, dtype=bf16)

    in_maps = []
    for c in range(NCORES):
        nk_cols = np.zeros((FIELDS * K, lrows), dtype=np.float32)
        gtiles = np.zeros((GPC * K, F), dtype=np.float32)
        for gl in range(GPC):
            g = c * GPC + gl
            if g >= FIELDS or counts[g] == 0:
                continue
            nk_cols[:, gl * pad : gl * pad + counts[g]] = (
                nfkT[:, off[g] : off[g + 1]]
            )
            gtiles[gl * K : (gl + 1) * K, :] = nfkT[g * K : (g + 1) * K, :]
        in_maps.append({
            "xt": _c(XT[:, c * BS : (c + 1) * BS].reshape(KT0, P_F, BS).transpose(1, 0, 2)),
            "w0": w0_dev,
            "w1w": w1_dev,
            "w2w": w2_dev,
            "outw": ow_dev,
            "w1lin": wl_dev,
            "b0": b0_dev, "b1": b1_dev, "b2": b2_dev,
            "nfkt_cols": _c(nk_cols.reshape(FIELDS, K, lrows).transpose(1, 0, 2)),
            "gtiles": _c(gtiles.reshape(GPC, K, F).transpose(1, 0, 2)),
            "halves": halves_dev,
        })

    res = run_bass_kernel_spmd(nc, in_maps, core_ids=list(range(NCORES)))
    global LAST_RESULT
    LAST_RESULT = res
    out = np.concatenate([r["out"].reshape(-1) for r in res.results])
    return out.astype(np.float32)


LAST_RESULT = None


if __name__ == "__main__":
    import importlib.util as _iu

    spec = _iu.spec_from_file_location("ref", "/root/problem/reference.py")
    ref = _iu.module_from_spec(spec)
    spec.loader.exec_module(ref)
    inp = {k: np.asarray(v) for k, v in ref.setup_inputs().items()}
    got = kernel(**inp)
    print("kernel out:", got[:8])



# revision 11
# speedup vs baseline: 1.0759x; 1.0759x over previous
"""DeepFFM Trainium2 kernel (8 NeuronCores, SPMD via bass/Tile) — v2.

Math (reference):
  linear      = X @ w1 + b
  S[i,j]      = <nfk[i, f2f[j], :], nfk[j, f2f[i], :]>   (symmetric, param-only)
  interaction = 0.5 * (x^T S x - sum_i S_ii x_i^2)  per batch row
  deep        = MLP(X) with relu layers
  out         = linear + interaction + deep

Strategy (v2 — no S AllGather):
  * Features are host-sorted by field. The 39 field groups are bin-packed
    into 8 bins of <=128 rows and <=NSLOT groups; core c builds ONLY the
    S rows of its bin, as ST[j, i] = S[i, j] tiles [125 x 128] via
    stacked-contraction matmuls: 3 groups share one 120-partition
    contraction, with zero-filled slots in the host-built R tensor
    encoding per-core group boundaries (SPMD-uniform program, per-core
    data).
  * Interaction partial: Y = ST^T @ XT over the FULL 4096 batch
    (XT loaded whole, 8.2MB), z_c[b] = 0.5*sum_i x_ib*Y_ib - 0.5*d_i x_ib^2
    (d = diag(S), host-computed from nfk). The per-core z partials
    [1,4096] fp32 are combined with a 16KB ReduceScatter (the fabric
    runs ~50GB/s — a 2.5MB S AllGather costs ~55us, 16KB is latency
    only and hides under the deep MLP).
  * Deep MLP + linear are batch-sharded (512 rows/core) exactly as v1.
  * All inputs are host-prearranged to exact SBUF layouts; bf16
    throughout with fp32 PSUM accumulation.
"""

import numpy as np

import concourse.bass as bass
import concourse.bacc as bacc
import concourse.mybir as mybir
import concourse.tile as tile
from concourse.bass_utils import run_bass_kernel_spmd

F32 = mybir.dt.float32
BF16 = mybir.dt.bfloat16

NCORES = 8
B = 4096
BS = B // NCORES          # batch rows per core
F = 1000                  # feature size
FIELDS = 39
K = 40                    # ffm embedding dim
D0, D1, D2 = 1024, 512, 256
KT0 = 8                   # k-chunks (125) over F
P_F = 125                 # partition chunk of F


def _pack_bins(counts, nslot):
    """Deterministic bin packing: 8 bins, <=128 rows, <=nslot groups."""
    import random
    rng = random.Random(0)
    base = list(np.argsort(-counts))

    def attempt(order):
        bins = [[] for _ in range(NCORES)]
        rows = [0] * NCORES
        for g in order:
            cand = [(128 - (rows[bb] + counts[g]), bb) for bb in range(NCORES)
                    if rows[bb] + counts[g] <= 128 and len(bins[bb]) < nslot]
            if not cand:
                return None
            _, bb = min(cand)
            bins[bb].append(int(g))
            rows[bb] += int(counts[g])
        return bins

    for trial in range(50000):
        order = base[:]
        if trial:
            rng.shuffle(order)
            order.sort(key=lambda g: -counts[g] + rng.uniform(-6, 6))
        bins = attempt(order)
        if bins:
            return bins
    return None


def _build_program(fsegs, nstack, bias_const, sim_single_core=False):
    """fsegs: list of (f, j0, j1) — global sorted-feature col range of each
    field (compile-time, baked into the program)."""
    from concourse.masks import make_identity
    nc = bacc.Bacc(None, num_devices=NCORES)

    xtf_h = nc.dram_tensor("xtf", [P_F, KT0, B], BF16, kind="ExternalInput")
    xts_h = nc.dram_tensor("xts", [P_F, KT0, BS], BF16, kind="ExternalInput")
    xloc_h = nc.dram_tensor("xloc", [128, B], BF16, kind="ExternalInput")
    rsl_h = nc.dram_tensor("rsl", [120, nstack, FIELDS, 128], BF16, kind="ExternalInput")
    gsl_h = nc.dram_tensor("gsl", [120, nstack, F], BF16, kind="ExternalInput")
    dneg_h = nc.dram_tensor("dneg", [128, 1], BF16, kind="ExternalInput")
    hv_h = nc.dram_tensor("halves", [128, 1], BF16, kind="ExternalInput")
    w0_h = nc.dram_tensor("w0", [P_F, 2, KT0, D1], BF16, kind="ExternalInput")
    w1_h = nc.dram_tensor("w1w", [128, 8, D1], BF16, kind="ExternalInput")
    w2_h = nc.dram_tensor("w2w", [128, 4, D2], BF16, kind="ExternalInput")
    ow_h = nc.dram_tensor("outw", [128, 2, 1], BF16, kind="ExternalInput")
    wl_h = nc.dram_tensor("w1lin", [P_F, KT0, 1], BF16, kind="ExternalInput")
    b0_h = nc.dram_tensor("b0", [128, 8], F32, kind="ExternalInput")
    b1_h = nc.dram_tensor("b1", [128, 4], F32, kind="ExternalInput")
    b2_h = nc.dram_tensor("b2", [128, 2], F32, kind="ExternalInput")
    out_h = nc.dram_tensor("out", [1, BS], F32, kind="ExternalOutput")

    with tile.TileContext(nc) as tc:
        with (
            tc.tile_pool(name="persist", bufs=1) as persist,
            tc.tile_pool(name="work", bufs=2) as work,
            tc.tile_pool(name="psum", bufs=1, space="PSUM") as psum,
            tc.tile_pool(name="dram", bufs=1, space="DRAM") as dram,
        ):
            # ---------------- loads ----------------
            # sync HW queue: rsl -> xtf[t0:3] -> w0 half0
            # scalar HW queue: gsl -> xtf[t3:8] -> w0 half1
            # gpsimd SW queue: smalls -> xts -> w2 -> xloc -> w1
            rsl_sb = persist.tile([120, nstack, FIELDS, 128], BF16)
            nc.sync.dma_start(out=rsl_sb, in_=rsl_h[:])
            gsl_sb = persist.tile([120, nstack, F], BF16)
            nc.scalar.dma_start(out=gsl_sb, in_=gsl_h[:])
            xtf_sb = persist.tile([P_F, KT0, B], BF16)
            nc.sync.dma_start(out=xtf_sb[:, 0:3, :], in_=xtf_h[:, 0:3, :])
            nc.scalar.dma_start(out=xtf_sb[:, 3:8, :], in_=xtf_h[:, 3:8, :])
            w0_sb = persist.tile([P_F, 2, KT0, D1], BF16)
            nc.sync.dma_start(out=w0_sb[:, 0, :, :], in_=w0_h[:, 0, :, :])
            nc.scalar.dma_start(out=w0_sb[:, 1, :, :], in_=w0_h[:, 1, :, :])

            halves = persist.tile([128, 1], BF16)
            nc.gpsimd.dma_start(out=halves, in_=hv_h[:])
            dneg_sb = persist.tile([128, 1], BF16)
            nc.gpsimd.dma_start(out=dneg_sb, in_=dneg_h[:])
            b0_sb = persist.tile([128, 8], F32)
            nc.gpsimd.dma_start(out=b0_sb, in_=b0_h[:])
            b1_sb = persist.tile([128, 4], F32)
            nc.gpsimd.dma_start(out=b1_sb, in_=b1_h[:])
            b2_sb = persist.tile([128, 2], F32)
            nc.gpsimd.dma_start(out=b2_sb, in_=b2_h[:])
            wl_sb = persist.tile([P_F, KT0, 1], BF16)
            nc.gpsimd.dma_start(out=wl_sb, in_=wl_h[:])
            ow_sb = persist.tile([128, 2, 1], BF16)
            nc.gpsimd.dma_start(out=ow_sb, in_=ow_h[:])
            xts_sb = persist.tile([P_F, KT0, BS], BF16)
            nc.gpsimd.dma_start(out=xts_sb, in_=xts_h[:])
            w2_sb = persist.tile([128, 4, D2], BF16)
            nc.gpsimd.dma_start(out=w2_sb, in_=w2_h[:])
            xloc_sb = persist.tile([128, B], BF16)
            nc.gpsimd.dma_start(out=xloc_sb, in_=xloc_h[:])
            w1_sb = persist.tile([128, 8, D1], BF16)
            nc.gpsimd.dma_start(out=w1_sb, in_=w1_h[:])

            # PE warm-up (HAM clock ramp)
            warm_sb = work.tile([128, 128], BF16, tag="warm", bufs=1)
            nc.vector.memset(warm_sb, 1.0)
            ps_w = psum.tile([128, 64], F32, tag="ps_tr", bufs=1)
            for _ in range(16):
                nc.tensor.matmul(
                    ps_w, lhsT=warm_sb[:, 0:128], rhs=warm_sb[:, 0:64],
                    start=True, stop=True,
                )

            # ---------------- S build: S_loc[i in mine, j] = [128, F] ---------
            # out[i, j in J_f] = sum_{(slot,k)} R[(s,k), f, i] * G[(s,k), j]
            # (R zero-filled outside slot(i)'s K-block selects the right field
            # pair). Then PE-transpose into the [125 j, 128 i] lhsT tiles the
            # interaction needs.
            ident = work.tile([128, 128], BF16, tag="ident", bufs=1)
            make_identity(nc, ident)
            ps_sl = psum.tile([128, F], F32, tag="ps_sl", bufs=1)
            for (f, j0, j1) in fsegs:
                for st in range(nstack):
                    nc.tensor.matmul(
                        ps_sl[:, j0:j1],
                        lhsT=rsl_sb[:, st, f, :],
                        rhs=gsl_sb[:, st, j0:j1],
                        start=(st == 0),
                        stop=(st == nstack - 1),
                    )
            s_loc = persist.tile([128, F], BF16)
            nc.vector.tensor_copy(s_loc, ps_sl)
            st_sb = persist.tile([P_F, KT0, 128], BF16)
            for t in range(KT0):
                ps_tr = psum.tile([P_F, 128], BF16, tag="ps_tr", bufs=1)
                nc.tensor.transpose(
                    ps_tr, s_loc[:, t * P_F:(t + 1) * P_F], ident
                )
                nc.vector.tensor_copy(st_sb[:, t, :], ps_tr)

            # keep the HAM clock warm across the xtf DMA-wait gap
            for _ in range(12):
                nc.tensor.matmul(
                    ps_w, lhsT=warm_sb[:, 0:128], rhs=warm_sb[:, 0:64],
                    start=True, stop=True,
                )

            # ---------------- interaction partials over the FULL batch -------
            z_sb = persist.tile([1, KT0, BS], F32)
            for bc in range(KT0):
                ps_y = psum.tile([128, BS], F32, tag="ps_mm", bufs=3)
                for t in range(KT0):
                    nc.tensor.matmul(
                        ps_y,
                        lhsT=st_sb[:, t, :],
                        rhs=xtf_sb[:, t, bc * BS:(bc + 1) * BS],
                        start=(t == 0),
                        stop=(t == KT0 - 1),
                    )
                zt = work.tile([128, BS], BF16, tag="zt")
                nc.vector.tensor_mul(zt, ps_y, xloc_sb[:, bc * BS:(bc + 1) * BS])
                x2 = work.tile([128, BS], BF16, tag="x2")
                nc.vector.tensor_mul(
                    x2,
                    xloc_sb[:, bc * BS:(bc + 1) * BS],
                    xloc_sb[:, bc * BS:(bc + 1) * BS],
                )
                ps_z = psum.tile([1, BS], F32, tag="ps_z", bufs=1)
                nc.tensor.matmul(ps_z, lhsT=halves, rhs=zt, start=True, stop=False)
                nc.tensor.matmul(ps_z, lhsT=dneg_sb, rhs=x2, start=False, stop=True)
                nc.vector.tensor_copy(z_sb[:, bc, :], ps_z)

            # ---------------- z ReduceScatter (16KB fp32) --------------------
            zin = dram.tile([1, KT0, BS], F32)
            zout = dram.tile([1, BS], F32)
            nc.gpsimd.dma_start(out=zin, in_=z_sb)
            if sim_single_core:
                nc.gpsimd.dma_start(out=zout, in_=zin[:, 0, :])
            else:
                nc.gpsimd.collective_compute(
                    "ReduceScatter",
                    mybir.AluOpType.add,
                    replica_groups=[list(range(NCORES))],
                    ins=[zin[:].opt()],
                    outs=[zout[:].opt()],
                )

            # ---------------- deep MLP (batch-sharded, as v1) -----------------
            h0_sb = persist.tile([128, 8, D1], BF16)
            ps_o = psum.tile([1, BS], F32, tag="ps_o", bufs=1)
            for mj in range(8):
                ps0 = psum.tile([128, BS], F32, tag="ps_mm", bufs=3)
                for t in range(KT0):
                    nc.tensor.matmul(
                        ps0,
                        lhsT=w0_sb[:, mj // 4, t, (mj % 4) * 128:(mj % 4 + 1) * 128],
                        rhs=xts_sb[:, t, :],
                        start=(t == 0),
                        stop=(t == KT0 - 1),
                    )
                nc.scalar.activation(
                    h0_sb[:, mj, :],
                    ps0,
                    mybir.ActivationFunctionType.Relu,
                    bias=b0_sb[:, mj:mj + 1],
                )
            h1_sb = persist.tile([128, 4, BS], BF16)
            for mj in range(4):
                ps1 = psum.tile([128, BS], F32, tag="ps_mm", bufs=3)
                for t in range(8):
                    nc.tensor.matmul(
                        ps1,
                        lhsT=w1_sb[:, t, mj * 128:(mj + 1) * 128],
                        rhs=h0_sb[:, t, :],
                        start=(t == 0),
                        stop=(t == 7),
                    )
                nc.scalar.activation(
                    h1_sb[:, mj, :],
                    ps1,
                    mybir.ActivationFunctionType.Relu,
                    bias=b1_sb[:, mj:mj + 1],
                )
            h2_sb = persist.tile([128, 2, BS], BF16)
            for mj in range(2):
                ps2 = psum.tile([128, BS], F32, tag="ps_mm", bufs=3)
                for t in range(4):
                    nc.tensor.matmul(
                        ps2,
                        lhsT=w2_sb[:, t, mj * 128:(mj + 1) * 128],
                        rhs=h1_sb[:, t, :],
                        start=(t == 0),
                        stop=(t == 3),
                    )
                nc.scalar.activation(
                    h2_sb[:, mj, :],
                    ps2,
                    mybir.ActivationFunctionType.Relu,
                    bias=b2_sb[:, mj:mj + 1],
                )
            # ps_o accumulation group: deep head + linear
            for t in range(2):
                nc.tensor.matmul(
                    ps_o, lhsT=ow_sb[:, t, :], rhs=h2_sb[:, t, :],
                    start=(t == 0), stop=False,
                )
            for t in range(KT0):
                nc.tensor.matmul(
                    ps_o, lhsT=wl_sb[:, t, :], rhs=xts_sb[:, t, :],
                    start=False, stop=(t == KT0 - 1),
                )

            # ---------------- final: + RS result + folded scalar bias --------
            zo_sb = persist.tile([1, BS], F32)
            nc.scalar.dma_start(out=zo_sb, in_=zout)
            out_sb = persist.tile([1, BS], F32)
            nc.vector.tensor_scalar_add(out_sb, ps_o, float(bias_const))
            nc.vector.tensor_add(out_sb, out_sb, zo_sb)
            nc.sync.dma_start(out=out_h[:], in_=out_sb)

    nc.compile()
    return nc


def kernel(X, w1, b, nfk, f2f, deepW0, deepB0, deepW1, deepB1, deepW2, deepB2,
           outW, outB, **_unused):
    import ml_dtypes
    bf16 = ml_dtypes.bfloat16

    X = np.ascontiguousarray(X, dtype=np.float32)
    w1 = np.asarray(w1, dtype=np.float32)
    b = np.asarray(b, dtype=np.float32)
    nfk = np.ascontiguousarray(nfk, dtype=np.float32)
    f2f = np.asarray(f2f, dtype=np.int64)
    deepW0 = np.ascontiguousarray(deepW0, dtype=np.float32)
    deepW1 = np.ascontiguousarray(deepW1, dtype=np.float32)
    deepW2 = np.ascontiguousarray(deepW2, dtype=np.float32)
    outW = np.ascontiguousarray(outW, dtype=np.float32)

    # ---- host-side layout transforms (index/permutation/cast work only) ----
    perm = np.argsort(f2f, kind="stable")
    counts = np.bincount(f2f, minlength=FIELDS).astype(int)
    off = np.zeros(FIELDS + 1, dtype=np.int64)
    off[1:] = np.cumsum(counts)
    f2fs = f2f[perm]

    nslot = 6
    bins = _pack_bins(counts, nslot)
    if bins is None:
        nslot = 7
        bins = _pack_bins(counts, nslot)
    assert bins is not None, "bin packing failed"
    nstack = (nslot + 2) // 3

    # per-field global col ranges, split at PSUM bank (512 fp32) boundaries
    fsegs = []
    for f in range(FIELDS):
        c0, c1 = int(off[f]), int(off[f + 1])
        while c0 < c1:
            nxt = min(c1, (c0 // 512 + 1) * 512)
            fsegs.append((f, c0, nxt))
            c0 = nxt

    XT = np.ascontiguousarray(X[:, perm].T)                     # [F, B]
    nfkp = nfk[perm]                                            # [F, FIELDS, K]
    W0p = np.ascontiguousarray(deepW0[perm])
    w1p = np.ascontiguousarray(w1[perm].reshape(F, 1))
    bias_const = float(np.float32(b[0]) + np.float32(outB[0]))

    nc = _build_program(fsegs, nstack, bias_const)

    def _c(a, dt=bf16):
        return np.ascontiguousarray(a).astype(dt)

    xtf_dev = _c(XT.reshape(KT0, P_F, B).transpose(1, 0, 2))
    w0_dev = _c(W0p.reshape(KT0, P_F, 2, D1).transpose(1, 2, 0, 3))
    w1_dev = _c(deepW1.reshape(8, 128, D1).transpose(1, 0, 2))
    w2_dev = _c(deepW2.reshape(4, 128, D2).transpose(1, 0, 2))
    ow_dev = _c(outW.reshape(2, 128, 1).transpose(1, 0, 2))
    wl_dev = _c(w1p.reshape(KT0, P_F, 1).transpose(1, 0, 2))
    b0_dev = np.ascontiguousarray(np.asarray(deepB0, np.float32).reshape(8, 128).T)
    b1_dev = np.ascontiguousarray(np.asarray(deepB1, np.float32).reshape(4, 128).T)
    b2_dev = np.ascontiguousarray(np.asarray(deepB2, np.float32).reshape(2, 128).T)
    halves_dev = np.full((128, 1), 0.5, dtype=bf16)

    in_maps = []
    for c in range(NCORES):
        groups = bins[c]
        myrows = np.concatenate(
            [np.arange(off[g], off[g + 1]) for g in groups]
        ).astype(np.int64)
        slot_of = np.concatenate(
            [np.full(counts[g], s, np.int64) for s, g in enumerate(groups)]
        )
        nmy = len(myrows)
        # R: [nstack, 120, FIELDS, 128] zero-padded slot encoding
        R = np.zeros((nstack, 120, FIELDS, 128), np.float32)
        for li in range(nmy):
            gi, s = myrows[li], slot_of[li]
            stk, sl = divmod(int(s), 3)
            R[stk, sl * K:(sl + 1) * K, :, li] = nfkp[gi].T
        G = np.zeros((nstack, 120, F), np.float32)
        for s, g in enumerate(groups):
            stk, sl = divmod(int(s), 3)
            G[stk, sl * K:(sl + 1) * K, :] = nfkp[:, g, :].T
        xloc = np.zeros((128, B), np.float32)
        xloc[:nmy] = XT[myrows]
        dvec = np.zeros((128, 1), np.float32)
        dvec[:nmy, 0] = np.sum(nfkp[myrows, f2fs[myrows], :] ** 2, axis=1)
        in_maps.append({
            "xtf": xtf_dev,
            "xts": _c(XT[:, c * BS:(c + 1) * BS].reshape(KT0, P_F, BS).transpose(1, 0, 2)),
            "xloc": _c(xloc),
            "rsl": _c(R.transpose(1, 0, 2, 3)),
            "gsl": _c(G.transpose(1, 0, 2)),
            "dneg": _c(-0.5 * dvec),
            "halves": halves_dev,
            "w0": w0_dev,
            "w1w": w1_dev,
            "w2w": w2_dev,
            "outw": ow_dev,
            "w1lin": wl_dev,
            "b0": b0_dev, "b1": b1_dev, "b2": b2_dev,
        })

    res = run_bass_kernel_spmd(nc, in_maps, core_ids=list(range(NCORES)))
    global LAST_RESULT
    LAST_RESULT = res
    out = np.concatenate([r["out"].reshape(-1) for r in res.results])
    return out.astype(np.float32)


LAST_RESULT = None


if __name__ == "__main__":
    import importlib.util as _iu

    spec = _iu.spec_from_file_location("ref", "/root/problem/reference.py")
    ref = _iu.module_from_spec(spec)
    spec.loader.exec_module(ref)
    inp = {k: np.asarray(v) for k, v in ref.setup_inputs().items()}
    got = kernel(**inp)
    print("kernel out:", got[:8])
